# revision 3
# baseline (speedup 1.0000x reference)
"""COMA loss kernel for Trainium2 (8 NeuronCores, data-parallel over batch).

TensorE-centric design.

Per core (B_local=16 -> 128 (b,a) rows): the big tensors are marshaled
to [128, 16384] bf16 tiles with partition p = bh*64 + n (n = action-dist
axis, bh = which half of the 128 (b,a) rows) and free column
c = ba'*256 + t (ba' = row % 64). All six per-(t,b,a) reductions over n
(sum_e, <e,q>, <e,logit>, and the three onehot gathers) then become
matmuls with block-ones stationaries, accumulated into one [12, 512]
PSUM tile per 512-column step. The onehot itself is produced by a
matmul that evaluates (n - act)^2 exactly in PSUM from 8 small
host-marshaled rows (exact bf16 integer decomposition), followed by a
ScalarE relu(1 - x). The DVE only runs the five bf16 elementwise
products feeding the matmuls, at its 2x bf16 mode.

Engine budget per core: DVE ~45us, ScalarE ~50us (exp + relu + PSUM
evac), PE ~35us, DMA ~40us (bf16 inputs). The lambda-return scan and
the final loss math run on [128, 256] f32 tiles exactly as in v1.
"""

import sys

for _p in ("/opt/trn_rl_repo",):
    if _p not in sys.path:
        sys.path.insert(0, _p)

import numpy as np
import ml_dtypes

import concourse.bass as bass
import concourse.bacc as bacc
import concourse.mybir as mybir
from concourse.bass_utils import run_bass_kernel_spmd
from concourse.tile import TileContext

T, B, A, N = 256, 128, 8, 64
M = 8                 # cores
BL = B // M           # local batch
BA = BL * A           # 128 rows
NB = BA // N          # 2 "bh" row-blocks stacked on the partition axis
F = (BA // NB) * T    # 16384 columns per core
CH = 2048             # columns per chunk (8 chunks)
NS = 512              # columns per matmul step
GAMMA, LAMBDA = 0.99, 0.95

F32 = mybir.dt.float32
BF16 = mybir.dt.bfloat16
NPBF16 = ml_dtypes.bfloat16

# The 12 reduction rows (2 per dot) land on PSUM/stg partitions chosen so
# each row lives in a different SBUF 4-partition block served by a
# different SDMA engine (port swizzle: blocks {0..7} -> even engines,
# {16..19} -> odd engines). The final row->[BA,T] scatter DMAs then
# stream from 12 distinct ports in parallel instead of one.
SROW = [4 * k if k < 8 else 32 + 4 * k for k in range(12)]
SP = 80  # stationary free dim / stg partition count (> max(SROW))


def build_program() -> bass.Bass:
    nc = bacc.Bacc("TRN2", target_bir_lowering=False, debug=False)

    lgT = nc.dram_tensor("lgT", [BA, F], BF16, kind="ExternalInput")
    qtT = nc.dram_tensor("qtT", [BA, F], BF16, kind="ExternalInput")
    tqT = nc.dram_tensor("tqT", [BA, F], BF16, kind="ExternalInput")
    ohT = nc.dram_tensor("ohT", [BA, F], BF16, kind="ExternalInput")
    ones6 = nc.dram_tensor("ones6", [BA, 6 * SP], BF16, kind="ExternalInput")
    wgt = nc.dram_tensor("wgt", [BA, T], F32, kind="ExternalInput")
    rwd = nc.dram_tensor("rwd", [BA, T], F32, kind="ExternalInput")
    out = nc.dram_tensor("out", [BA, 3], F32, kind="ExternalOutput")

    AX = mybir.AxisListType.X
    OP = mybir.AluOpType
    AF = mybir.ActivationFunctionType

    with TileContext(nc) as tc:
        with (
            tc.tile_pool(name="per", bufs=1) as per,
            tc.tile_pool(name="inp", bufs=2) as inp,
            tc.tile_pool(name="scr", bufs=3) as scr,
            tc.tile_pool(name="psq", bufs=4, space="PSUM") as psq,
        ):
            # ---- constants / small inputs -------------------------------
            s_ones = per.tile([BA, 6 * SP], BF16)
            nc.sync.dma_start(out=s_ones[:], in_=ones6[:])
            w_t = per.tile([BA, T], F32)
            nc.gpsimd.dma_start(out=w_t[:], in_=wgt[:])
            r_t = per.tile([BA, T], F32)
            nc.gpsimd.dma_start(out=r_t[:], in_=rwd[:])

            # staging for the 12 reduction rows (2 per dot: bh0, bh1),
            # spread over partitions SROW for parallel scatter streams
            stg = per.tile([SP, F], F32)

            # final [BA, T] per-dot tiles, filled by the tail scatter
            sum_e = per.tile([BA, T], F32)
            dot_eq = per.tile([BA, T], F32)
            dot_el = per.tile([BA, T], F32)
            q_tk = per.tile([BA, T], F32)
            tq_tk = per.tile([BA, T], F32)
            l_tk = per.tile([BA, T], F32)
            dests = [sum_e, dot_eq, dot_el, q_tk, tq_tk, l_tk]
            half = BA // NB

            # ---- stage 1: chunked column sweep --------------------------
            chunk_sizes = [CH] * (F // CH)
            c0 = 0
            for ck, ch in enumerate(chunk_sizes):
                lg = inp.tile([BA, CH], BF16, tag="lg", name="lg")[:, :ch]
                qt = inp.tile([BA, CH], BF16, tag="qt", name="qt")[:, :ch]
                tq = inp.tile([BA, CH], BF16, tag="tq", name="tq")[:, :ch]
                oh = inp.tile([BA, CH], BF16, tag="oh", name="oh")[:, :ch]
                nc.sync.dma_start(out=lg[:], in_=lgT[:, c0 : c0 + ch])
                nc.sync.dma_start(out=qt[:], in_=qtT[:, c0 : c0 + ch])
                nc.sync.dma_start(out=oh[:], in_=ohT[:, c0 : c0 + ch])
                nc.sync.dma_start(out=tq[:], in_=tqT[:, c0 : c0 + ch])

                e = scr.tile([BA, CH], BF16, tag="e", name="e")[:, :ch]
                nc.scalar.activation(out=e[:], in_=lg[:], func=AF.Exp)

                # five bf16 products (DVE 2x mode; one on GpSimd)
                p_eq = scr.tile([BA, CH], BF16, tag="p_eq", name="p_eq")[:, :ch]
                p_el = scr.tile([BA, CH], BF16, tag="p_el", name="p_el")[:, :ch]
                g_q = scr.tile([BA, CH], BF16, tag="g_q", name="g_q")[:, :ch]
                g_tq = scr.tile([BA, CH], BF16, tag="g_tq", name="g_tq")[:, :ch]
                g_lg = scr.tile([BA, CH], BF16, tag="g_lg", name="g_lg")[:, :ch]
                nc.vector.tensor_mul(p_eq[:], e[:], qt[:])
                nc.vector.tensor_mul(p_el[:], e[:], lg[:])
                nc.vector.tensor_mul(g_q[:], oh[:], qt[:])
                nc.vector.tensor_mul(g_tq[:], oh[:], tq[:])
                nc.vector.tensor_mul(g_lg[:], oh[:], lg[:])
                movings = [e, p_eq, p_el, g_q, g_tq, g_lg]

                # six dot-reductions per 512-col step, accumulated into
                # [SP, NS] PSUM rows (SROW[2d] = bh0, SROW[2d+1] = bh1),
                # evac'd by ScalarE into the f32 staging tile.
                nj = max(1, ch // (2 * NS))
                for j in range(nj):
                    dots = psq.tile([SP, 2, NS], F32, tag="dots")
                    for d, mv in enumerate(movings):
                        for s in range(min(2, ch // NS)):
                            cs = 2 * NS * j + NS * s
                            nc.tensor.matmul(
                                dots[:, s, :],
                                s_ones[:, SP * d : SP * (d + 1)],
                                mv[:, cs : cs + NS],
                                start=(d == 0),
                                stop=(d == 5),
                                skip_group_check=True,
                            )
                    ce = min(ch, 2 * NS * (j + 1))
                    nc.scalar.copy(
                        out=stg[:, c0 + 2 * NS * j : c0 + ce],
                        in_=dots[:, : (ce - 2 * NS * j + NS - 1) // NS, :].rearrange(
                            "p a b -> p (a b)"
                        ),
                    )
                c0 += ch

            # ---- tail scatter: 12 parallel row streams, 2 DMA queues ----
            # Order rows so the longest stage-2 chains unblock first.
            order = [0, 4, 3, 1, 5, 2]  # sum_e, tq_tk, q_tk, dot_eq, l_tk, dot_el
            for i, d in enumerate(order):
                for bh in range(NB):
                    eng = nc.sync if (2 * i + bh) % 2 == 0 else nc.gpsimd
                    eng.dma_start(
                        out=dests[d][half * bh : half * (bh + 1), :],
                        in_=stg[
                            SROW[2 * d + bh] : SROW[2 * d + bh] + 1, :
                        ].rearrange("p (b t) -> p b t", b=half),
                    )

            # ---- stage 2: per-(t,ba) scalar math on [128, T] ------------
            # Three parallel branches: lambda-return scan (DVE, longest),
            # policy (DVE), entropy (GpSimd). 1/sum_e comes from ScalarE
            # as exp(-z) to dodge the slow DVE reciprocal.
            gl = per.tile([BA, 1], F32)
            nc.vector.memset(gl[:], GAMMA * LAMBDA)

            # scan branch
            d = per.tile([BA, T - 1], F32)
            nc.vector.scalar_tensor_tensor(
                out=d[:],
                in0=tq_tk[:, 1:T],
                scalar=GAMMA * (1.0 - LAMBDA),
                in1=r_t[:, 0 : T - 1],
                op0=OP.mult,
                op1=OP.add,
            )
            ret = per.tile([BA, T - 1], F32)
            nc.vector.tensor_tensor_scan(
                out=ret[:, ::-1],
                data0=gl[:].to_broadcast([BA, T - 1]),
                data1=d[:, ::-1],
                initial=tq_tk[:, T - 1 : T],
                op0=OP.mult,
                op1=OP.add,
            )
            qd = per.tile([BA, T - 1], F32)
            nc.vector.tensor_tensor(
                out=qd[:], in0=ret[:], in1=q_tk[:, 0 : T - 1], op=OP.subtract
            )
            nc.vector.tensor_mul(qd[:], qd[:], qd[:])
            nc.vector.tensor_mul(qd[:], qd[:], w_t[:, 0 : T - 1])

            z = per.tile([BA, T], F32)  # logsumexp
            nc.scalar.activation(out=z[:], in_=sum_e[:], func=AF.Ln)
            rs = per.tile([BA, T], F32)  # 1/sum_e = exp(-z)
            nc.scalar.activation(out=rs[:], in_=z[:], func=AF.Exp, scale=-1.0)

            # policy branch (DVE)
            logp = per.tile([BA, T], F32)
            nc.vector.tensor_tensor(out=logp[:], in0=l_tk[:], in1=z[:], op=OP.subtract)
            bl = per.tile([BA, T], F32)  # baseline = dot_eq / sum_e
            nc.vector.tensor_mul(bl[:], dot_eq[:], rs[:])
            adv = per.tile([BA, T], F32)
            nc.vector.tensor_tensor(out=adv[:], in0=q_tk[:], in1=bl[:], op=OP.subtract)
            pol = per.tile([BA, T], F32)  # logp * adv * w
            nc.vector.tensor_mul(pol[:], logp[:], adv[:])
            nc.vector.tensor_mul(pol[:], pol[:], w_t[:])

            # entropy branch: entropy = z - dot_el / sum_e
            ent = per.tile([BA, T], F32)
            nc.vector.tensor_mul(ent[:], dot_el[:], rs[:])
            nc.vector.tensor_tensor(out=ent[:], in0=z[:], in1=ent[:], op=OP.subtract)
            entw = per.tile([BA, T], F32)
            nc.vector.tensor_mul(entw[:], ent[:], w_t[:])

            partials = per.tile([BA, 3], F32)
            nc.vector.reduce_sum(out=partials[:, 1:2], in_=qd[:], axis=AX)
            nc.vector.reduce_sum(out=partials[:, 0:1], in_=pol[:], axis=AX)
            nc.vector.reduce_sum(out=partials[:, 2:3], in_=entw[:], axis=AX)
            nc.sync.dma_start(out=out[:], in_=partials[:])

    return nc


def _make_stationaries():
    ones6 = np.zeros((BA, 6 * SP), dtype=np.float32)
    for d in range(6):
        ones6[0:64, SP * d + SROW[2 * d]] = 1.0
        ones6[64:128, SP * d + SROW[2 * d + 1]] = 1.0
    return ones6.astype(NPBF16)


_ONES6 = _make_stationaries()
_IOTA64 = np.arange(64, dtype=np.int64)[:, None]


def _big_to_tile(x):
    """[T, 16, A, N] f32 slice -> [128, 16384] bf16 with p = bh*64 + n,
    c = ba'*T + t."""
    z = x.reshape(T, BA, N).transpose(2, 1, 0)          # [n, ba, t]
    z = z.reshape(N, NB, BA // NB, T).transpose(1, 0, 2, 3)  # [bh, n, ba', t]
    return np.ascontiguousarray(z.reshape(BA, F)).astype(NPBF16)


def make_in_maps(logit, action, q_value, target_q_value, reward, weight):
    """Shard + marshal full inputs into per-core input dicts."""
    logit = np.asarray(logit, np.float32)
    q_value = np.asarray(q_value, np.float32)
    target_q_value = np.asarray(target_q_value, np.float32)
    action = np.asarray(action)
    reward = np.asarray(reward, np.float32)
    weight = np.asarray(weight, np.float32)

    in_maps = []
    for r in range(M):
        bs, be = r * BL, (r + 1) * BL
        act = action[:, bs:be].reshape(T, BA).T.astype(np.int64)  # [ba, t]
        half = BA // NB
        # onehot of the action index, in the same [p=bh*64+n, c=ba'*T+t]
        # tile layout as the big tensors (index expansion, exact in bf16)
        ohT = np.empty((BA, F), dtype=NPBF16)
        for bh in range(NB):
            a = act[half * bh : half * (bh + 1), :].reshape(F)  # c = ba'*T + t
            ohT[64 * bh : 64 * bh + 64, :] = (_IOTA64 == a[None, :]).astype(NPBF16)

        in_maps.append(
            {
                "lgT": _big_to_tile(logit[:, bs:be]),
                "qtT": _big_to_tile(q_value[:, bs:be]),
                "tqT": _big_to_tile(target_q_value[:, bs:be]),
                "ohT": ohT,
                "ones6": _ONES6,
                "wgt": np.ascontiguousarray(weight[:, bs:be].reshape(T, BA).T),
                "rwd": np.ascontiguousarray(
                    np.repeat(reward[:, bs:be], A, axis=1).T
                ),
            }
        )
    return in_maps


def combine_partials(partials_per_core):
    """[M][128,3] partial sums -> the three scalar losses."""
    s = np.stack(partials_per_core).astype(np.float64).sum(axis=(0, 1))
    policy_loss = np.float32(-s[0] / (T * B * A))
    q_value_loss = np.float32(s[1] / ((T - 1) * B * A))
    entropy_loss = np.float32(s[2] / (T * B * A))
    return policy_loss, q_value_loss, entropy_loss


_program_cache = {}


def _get_program() -> bass.Bass:
    if "nc" not in _program_cache:
        nc = build_program()
        nc.finalize()
        _program_cache["nc"] = nc
    return _program_cache["nc"]


def kernel(logit, action, q_value, target_q_value, reward, weight):
    nc = _get_program()
    in_maps = make_in_maps(logit, action, q_value, target_q_value, reward, weight)
    res = run_bass_kernel_spmd(nc, in_maps, list(range(M))).results
    return combine_partials([np.asarray(res[i]["out"]) for i in range(M)])
